# revision 14
# baseline (speedup 1.0000x reference)
"""Causal attention (dense transformer block) on 8 Trainium2 NeuronCores.

Problem: x (4, 256, 64, 64) fp32; 1x1-conv q/kv projections; 8-head causal
attention over S = 64*64 = 4096 flattened pixels (head_dim 32); output
projection.  Full inputs in, full output out.

Sharding: 8 cores = 4 batches x 2 head-groups (4 heads each).  Each core
computes q/k/v projections for its head group, flash-style causal attention
(scores kept transposed: k-positions on partitions, q-positions on free dim,
so softmax denominators come out of the AV matmul via an appended ones
column), and a partial output projection.  Host sums the two head-group
partials per batch and adds the output bias.

Engine split: PE does all matmuls (f32r, full rate at moving>=256); ScalarE
does ONLY the softmax exp (it is the 2nd-busiest engine and exp alone is
~230us); DVE does projections' PSUM evacuation + bias, causal masks, and
softmax normalization; Pool broadcasts the reciprocal rows; denominator rows
move PSUM->SBUF by DMA.  Rows run j-descending so the per-row normalization
chain hides under long rows.
"""

import math
from contextlib import ExitStack

import numpy as np

import concourse.bass as bass
import concourse.tile as tile
from concourse import bacc, mybir

N_CORES = 8
N, C, HH, WW = 4, 256, 64, 64
S = HH * WW            # 4096
E = 256                # q/k width
O = 256                # v/out width
H = 8                  # heads
HD = E // H            # 32 head dim
HG = 4                 # heads per core
P = 128                # partitions
QC = 512               # q-chunk (matmul moving free dim)
KT = 128               # k-tile (contraction block for AV)
NQ = S // QC           # 8 q-chunks
G = 4                  # windows per emission group

F32 = mybir.dt.float32
F32R = mybir.dt.float32r
BF16 = mybir.dt.bfloat16

# bf16 for matmul STATIONARY operands only (k-tiles, v_st, wq/wk/wp): on
# HW, LDWEIGHTS streams 2 cols/cycle for 16-bit vs ~0.5 for f32r.  Moving
# operands stay f32r (full-rate at width>=256), so PE row cost is unchanged.
STAT_BF16 = False
SDT = BF16 if STAT_BF16 else F32R


def build_kernel(reps=1):
    nc = bacc.Bacc("TRN2", target_bir_lowering=False, debug=False,
                   num_devices=N_CORES)

    # Per-core inputs (same shapes on every core, different data).
    xf = nc.dram_tensor("xf", (C, S), F32, kind="ExternalInput").ap()
    wqT = nc.dram_tensor("wqT", (C, P), F32, kind="ExternalInput").ap()
    wkT = nc.dram_tensor("wkT", (C, P), F32, kind="ExternalInput").ap()
    wvT = nc.dram_tensor("wvT", (C, O), F32, kind="ExternalInput").ap()
    wpT = nc.dram_tensor("wpT", (2, P, P), F32, kind="ExternalInput").ap()
    bq = nc.dram_tensor("bq", (P, 1), F32, kind="ExternalInput").ap()
    bk = nc.dram_tensor("bk", (P, 1), F32, kind="ExternalInput").ap()
    bv = nc.dram_tensor("bv", (1, P), F32, kind="ExternalInput").ap()
    masks = nc.dram_tensor("masks", (4, P, QC), F32, kind="ExternalInput").ap()
    out = nc.dram_tensor("out", (O, S), F32, kind="ExternalOutput").ap()

    with tile.TileContext(nc) as tc:
        with ExitStack() as ctx:
            _emit(ctx, tc, nc, xf, wqT, wkT, wvT, wpT, bq, bk, bv, masks, out,
                  reps=reps)

    nc.compile()
    return nc


def _row_units(j):
    """Causal k-tile units for one (head, j) row: (kt, c0, width).

    Clean tiles (kt < 4j) are full 512.  Diagonal tiles shrink to their
    causally-valid columns where f32r still runs full rate (width >= 256);
    the 384-wide t1 unit goes last so its partially-used psum bank ends the
    row and the exp span stays dense.
    """
    units = [(kt, 0, QC) for kt in range(4 * j)]
    units.append((4 * j, 0, 512))
    units.append((4 * j + 2, 256, 256))
    units.append((4 * j + 3, 256, 256))
    units.append((4 * j + 1, 128, 384))
    return units


def _row_windows(j):
    """Pack a row's units into psum windows of <= 2 banks.

    Returns [(units_with_offsets, exp_width)] where units are
    (kt, c0, off, width) and matmul psum writes never cross a 512-col bank.
    """
    units = _row_units(j)
    # assign to banks: greedy, each bank holds units summing to <= 512
    banks = []
    cur, used = [], 0
    for (kt, c0, w) in units:
        if used + w > QC:
            banks.append((cur, used))
            cur, used = [], 0
        cur.append((kt, c0, used, w))
        used += w
    banks.append((cur, used))

    windows = []
    for b0 in range(0, len(banks), 2):
        grp = banks[b0:b0 + 2]
        wunits = []
        exp_w = 0
        for bi, (bunits, used) in enumerate(grp):
            for (kt, c0, off, w) in bunits:
                wunits.append((kt, c0, bi * QC + off, w))
            exp_w = bi * QC + used
        windows.append((wunits, exp_w))
    return windows


def _emit(ctx, tc, nc, xf, wqT, wkT, wvT, wpT, bq, bk, bv, masks, out,
          reps=1):
    scale = 1.0 / math.sqrt(HD)
    Exp = mybir.ActivationFunctionType.Exp

    consts = ctx.enter_context(tc.tile_pool(name="consts", bufs=1))
    qk_ps = ctx.enter_context(tc.tile_pool(name="qk_ps", bufs=2, space="PSUM"))
    av_ps = ctx.enter_context(tc.tile_pool(name="av_ps", bufs=2, space="PSUM"))
    bal_ps = ctx.enter_context(tc.tile_pool(name="bal_ps", bufs=2, space="PSUM"))
    work = ctx.enter_context(tc.tile_pool(name="work", bufs=6))
    norm = ctx.enter_context(tc.tile_pool(name="norm", bufs=2))
    tmp = ctx.enter_context(tc.tile_pool(name="tmp", bufs=1))

    # ---- load constants / weights -------------------------------------
    # DMA-loaded data cannot feed FP32r matmuls directly; a compute-engine
    # copy with float32r output performs the required rounding.
    wq_ld = tmp.tile([P, 2, P], F32, tag="w")
    nc.sync.dma_start(out=wq_ld, in_=wqT.rearrange("(c p) m -> p c m", p=P))
    wq_sb = consts.tile([P, 2, P], SDT)
    nc.vector.tensor_copy(wq_sb, wq_ld)
    wk_ld = tmp.tile([P, 2, P], F32, tag="w")
    nc.sync.dma_start(out=wk_ld, in_=wkT.rearrange("(c p) m -> p c m", p=P))
    wk_sb = consts.tile([P, 2, P], SDT)
    nc.vector.tensor_copy(wk_sb, wk_ld)
    wv_ld = tmp.tile([P, 2, O], F32, tag="w")
    nc.sync.dma_start(out=wv_ld, in_=wvT.rearrange("(c p) m -> p c m", p=P))
    wv_sb = consts.tile([P, 2, O], F32R)
    nc.vector.tensor_copy(wv_sb, wv_ld)
    wp_ld = tmp.tile([P, 2, P], F32, tag="w")
    nc.sync.dma_start(out=wp_ld, in_=wpT.rearrange("m p n -> p m n"))
    wp_sb = consts.tile([P, 2, P], SDT)
    nc.vector.tensor_copy(wp_sb, wp_ld)
    # small tensors (biases, masks) go on the SP queue ahead of the big x
    # transfer so nothing downstream waits on queue order
    bq_sb = consts.tile([P, 1], F32)
    nc.sync.dma_start(out=bq_sb, in_=bq)
    bk_sb = consts.tile([P, 1], F32)
    nc.sync.dma_start(out=bk_sb, in_=bk)
    bv_row = consts.tile([1, P], F32)
    nc.sync.dma_start(out=bv_row, in_=bv)
    mask_sb = consts.tile([P, 4, QC], F32)
    nc.sync.dma_start(out=mask_sb, in_=masks.rearrange("m p q -> p m q"))
    # sliced load+round so the first projection matmuls start after one
    # 512-col slice instead of the whole 4 MB x transfer; Pool queue (idle,
    # 25ns dispatch) so the x slices don't serialize behind the SP queue
    x_ld = tmp.tile([P, 2, S], F32, tag="big")  # xf as two 128-row chunks
    x_sb = consts.tile([P, 2, S], F32R)
    xr = xf.rearrange("(c p) s -> p c s", p=P)
    for sl in range(NQ):
        nc.sync.dma_start(out=x_ld[:, :, bass.ts(sl, QC)],
                          in_=xr[:, :, bass.ts(sl, QC)])
        nc.vector.tensor_copy(x_sb[:, :, bass.ts(sl, QC)],
                              x_ld[:, :, bass.ts(sl, QC)])
    bv_bc = consts.tile([P, P], F32)            # bv broadcast down partitions
    nc.gpsimd.partition_broadcast(bv_bc, bv_row)

    # v_st[h]: (128 kpos, 34) per k-tile: cols 0:32 = v, col 32 = 1.0 (the
    # softmax-denominator ones column).  Ones written once, outside reps.
    v_st = [consts.tile([P, S // KT, 34], SDT, name=f"v_st{h}")
            for h in range(HG)]
    ones2 = consts.tile([P, 2], F32)
    nc.vector.memset(ones2, 1.0)
    ones_bc = bass.AP(tensor=ones2.tensor, offset=ones2.offset,
                      ap=[ones2.ap[0], [0, S // KT], ones2.ap[1]])
    for h in range(HG):
        nc.vector.tensor_copy(v_st[h][:, :, 32:34], ones_bc)

    # On-device repeat loop for timing runs (reps>1): the whole compute
    # phase re-executes; consts/DMA loads stay outside.
    if reps > 1:
        loop_cm = tc.For_i(0, reps, 1)
        loop_cm.__enter__()

    qT = consts.tile([P, S], F32R)               # 4 heads x 32 dims on parts
    kT = consts.tile([P, S], SDT)
    q3 = tmp.tile([HD, S], F32R, tag="big")
    k3 = consts.tile([HD, S], SDT, name="k3")
    outn = consts.tile([P, S], F32R)             # normalized out^T, 4h x 32dv

    # ---- interleaved projection units (ballast for the window stream) ---
    # Attention row (h, j) needs q/k projection columns jj <= j and v tiles
    # st <= 4j+3, so projections stream into the attention pipeline instead
    # of serializing up front.  need[0] of each unit = first j that uses it.

    bq_bc = bass.AP(tensor=bq_sb.tensor, offset=bq_sb.offset,
                    ap=[bq_sb.ap[0], [0, QC]])
    bk_bc = bass.AP(tensor=bk_sb.tensor, offset=bk_sb.offset,
                    ap=[bk_sb.ap[0], [0, QC]])

    def emit_qkproj(jj):
        for dst, w_sb, b_bc, r in ((qT, wq_sb, bq_bc, 0), (kT, wk_sb, bk_bc, 1)):
            ps = bal_ps.tile([P, QC], F32, tag="bal")
            for cc in range(2):
                nc.tensor.matmul(ps[:, 0:QC], w_sb[:, cc, :],
                                 x_sb[:, cc, bass.ts(jj, QC)],
                                 start=(cc == 0), stop=(cc == 1))
            nc.vector.tensor_add(dst[:, bass.ts(jj, QC)], ps[:, 0:QC], b_bc)
            # head 3 cannot feed PE weights from partition 96; partition-0 copy
            h3 = q3 if r == 0 else k3
            nc.vector.tensor_copy(h3[:, bass.ts(jj, QC)],
                                  dst[3 * HD:4 * HD, bass.ts(jj, QC)])

    def emit_vproj(st0):
        # position-major v, two k-tiles of 128 positions per psum take;
        # N=256 (all 8 heads) keeps f32r full-rate; wvT pre-rotated so our
        # 128 cols sit at 0:128 — the 256:512 half holds the other group's
        # (unused) columns, overwritten by the second k-tile's output.
        ps = bal_ps.tile([P, QC], F32, tag="bal")
        for i, st in enumerate((st0, st0 + 1)):
            for cc in range(2):
                nc.tensor.matmul(ps[:, i * O:(i + 1) * O],
                                 x_sb[:, cc, bass.ts(st, KT)],
                                 wv_sb[:, cc, :],
                                 start=(cc == 0), stop=(cc == 1))
        for i in range(2):
            for h in range(HG):
                nc.vector.tensor_add(v_st[h][:, st0 + i, 0:32],
                                     ps[:, i * O + h * HD:
                                        i * O + (h + 1) * HD],
                                     bv_bc[:, h * HD:(h + 1) * HD])

    def emit_outproj(jj):
        for m in range(2):
            ps = bal_ps.tile([P, QC], F32, tag="bal")
            nc.tensor.matmul(ps[:, 0:QC], wp_sb[:, m, :],
                             outn[:, bass.ts(jj, QC)], start=True, stop=True)
            ob = work.tile([P, QC], F32, tag="ob", bufs=2)
            nc.vector.tensor_copy(ob, ps[:, 0:QC])
            nc.sync.dma_start(
                out=out.rearrange("(m p) s -> p m s", p=P)[:, m,
                                                           bass.ts(jj, QC)],
                in_=ob)

    ballast = []
    for jj in range(NQ):
        ballast.append((jj, emit_qkproj, jj))
        for st in range(4 * jj, 4 * jj + 4, 2):
            ballast.append((jj, emit_vproj, st))
    bpos = 0

    # ---- attention ------------------------------------------------------
    # Rows (h, j) run j-ascending.  Windows emit in groups of G: G windows
    # of QK+exp+mask, then those windows' AV matmuls — long PE runs of one
    # stationary shape, with exp overlapping inside the group.  Two AV
    # accumulators pack per psum bank (partition offsets 0 and 64), so four
    # rows can be in flight on two banks and the per-row normalization
    # chain (den copy -> recip -> broadcast -> multiply) hides under the
    # following rows.
    flat = []                                    # (h, j, wunits, exp_w, u0, nu)
    for j in range(NQ):
        for h in range(HG):
            wins = _row_windows(j)
            nu = sum(len(wu) for (wu, _) in wins)
            u0 = 0
            for (wu, exp_w) in wins:
                flat.append((h, j, wu, exp_w, u0, nu))
                u0 += len(wu)
    groups = [flat[g0:g0 + G] for g0 in range(0, len(flat), G)]

    av_tiles = {}
    out_queue = []
    norm_done = set()

    def emit_qk_grp(grp):
        ess = []
        for (h, j, wunits, exp_w, u0, nu) in grp:
            qh = qT[h * HD:(h + 1) * HD, :] if h < 3 else q3
            kh = kT[h * HD:(h + 1) * HD, :] if h < 3 else k3
            ps = qk_ps.tile([P, 2 * QC], F32, tag="qk", name="ps")
            for (kt, c0, off, w) in wunits:
                nc.tensor.matmul(ps[:, off:off + w],
                                 kh[:, bass.ts(kt, KT)],
                                 qh[:, j * QC + c0:(j + 1) * QC],
                                 start=True, stop=True)
            es = work.tile([P, 2 * QC], F32R, tag="es", name="es", bufs=5)
            nc.scalar.activation(es[:, 0:exp_w], ps[:, 0:exp_w],
                                 Exp, scale=scale)
            # causal mask post-exp (multiply by 0/1): off the QK->exp path
            for (kt, c0, off, w) in wunits:
                t = kt - 4 * j
                if t >= 0:
                    nc.vector.tensor_mul(es[:, off:off + w],
                                         es[:, off:off + w],
                                         mask_sb[:, t, c0:QC])
            ess.append(es)
        return ess

    def emit_av_grp(grp, ess):
        for es, (h, j, wunits, exp_w, u0, nu) in zip(ess, grp):
            if (h, j) not in av_tiles:
                av_tiles[(h, j)] = av_ps.tile([P, QC], F32, tag="av",
                                              name="av")
            av = av_tiles[(h, j)]
            b = 0
            for ui, (kt, c0, off, w) in enumerate(wunits):
                nc.tensor.matmul(av[b:b + 33, c0:QC],
                                 v_st[h][:, kt, 0:33],
                                 es[:, off:off + w],
                                 start=(u0 + ui == 0),
                                 stop=(u0 + ui == nu - 1))
            if u0 + len(wunits) == nu:
                # row complete: denominator -> reciprocal -> broadcast ->
                # normalize straight out of PSUM.
                den = norm.tile([1, QC], F32, tag="den", name="den")
                nc.vector.tensor_copy(den, av[b + 32:b + 33, :])
                recip = norm.tile([1, QC], F32, tag="recip", name="recip")
                rscr = norm.tile([1, QC], F32, tag="rscr", name="rscr")
                nc.vector.reciprocal_approx_accurate(recip, den, rscr)
                rbc = norm.tile([HD, QC], F32, tag="rbc", name="rbc")
                nc.gpsimd.partition_broadcast(rbc, recip)
                nc.vector.tensor_mul(
                    outn[h * HD:(h + 1) * HD, bass.ts(j, QC)],
                    av[b:b + 32, :], rbc)
                del av_tiles[(h, j)]
                norm_done.add((h, j))
                if all((hh, j) in norm_done for hh in range(HG)):
                    out_queue.append(j)

    for gi, grp in enumerate(groups):
        need = max(j for (_, j, _, _, _, _) in grp)
        while bpos < len(ballast) and ballast[bpos][0] <= need and \
                ballast[bpos][1] is emit_qkproj:
            _, fn, arg = ballast[bpos]
            fn(arg)
            bpos += 1
        ess = emit_qk_grp(grp)
        while bpos < len(ballast) and ballast[bpos][0] <= need:
            _, fn, arg = ballast[bpos]
            fn(arg)
            bpos += 1
        emit_av_grp(grp, ess)
        for _ in range(2):
            if bpos < len(ballast):
                _, fn, arg = ballast[bpos]
                fn(arg)
                bpos += 1
        npop = 2 if len(groups) - gi < 10 else 1
        for _ in range(npop):
            if out_queue:
                emit_outproj(out_queue.pop(0))

    while bpos < len(ballast):
        _, fn, arg = ballast[bpos]
        fn(arg)
        bpos += 1
    while out_queue:
        emit_outproj(out_queue.pop(0))

    if reps > 1:
        loop_cm.__exit__(None, None, None)


_BUILT = {}


def _get_built(reps=1):
    if reps not in _BUILT:
        _BUILT[reps] = build_kernel(reps)
    return _BUILT[reps]


def make_in_maps(x, Wq, bq, Wkv, bkv, Wp, bp):
    x = np.asarray(x, dtype=np.float32)
    Wq = np.asarray(Wq, dtype=np.float32)
    bq = np.asarray(bq, dtype=np.float32)
    Wkv = np.asarray(Wkv, dtype=np.float32)
    bkv = np.asarray(bkv, dtype=np.float32)
    Wp = np.asarray(Wp, dtype=np.float32)

    Wk, Wv = Wkv[:E], Wkv[E:]
    bk_, bv_ = bkv[:E], bkv[E:]

    # causal masks in transposed-score orientation (kpos partition, qpos free)
    kk = np.arange(P)[:, None]
    qq = np.arange(QC)[None, :]
    mask_np = np.stack([
        (qq >= d0 + kk).astype(np.float32)
        for d0 in (0, 128, 256, 384)])

    in_maps = []
    for c in range(N_CORES):
        n, hg = c // 2, c % 2
        rows = slice(hg * P, (hg + 1) * P)
        # rotate wvT columns so this core's 128 head columns sit at 0:128
        wvT_c = np.ascontiguousarray(np.roll(Wv.T, -hg * P, axis=1))
        in_maps.append({
            "xf": np.ascontiguousarray(x[n].reshape(C, S)),
            "wqT": np.ascontiguousarray(Wq[rows].T),
            "wkT": np.ascontiguousarray(Wk[rows].T),
            "wvT": wvT_c,
            "wpT": np.ascontiguousarray(
                Wp[:, rows].reshape(2, P, P).transpose(0, 2, 1)),
            "bq": np.ascontiguousarray(bq[rows, None]),
            "bk": np.ascontiguousarray(bk_[rows, None]),
            "bv": np.ascontiguousarray(bv_[None, rows]),
            "masks": mask_np,
        })
    return in_maps


def kernel(x, Wq, bq, Wkv, bkv, Wp, bp, n_heads):
    assert int(n_heads) == H
    bp = np.asarray(bp, dtype=np.float32)

    from concourse.bass_utils import run_bass_kernel_spmd

    nc = _get_built()
    in_maps = make_in_maps(x, Wq, bq, Wkv, bkv, Wp, bp)

    res = run_bass_kernel_spmd(nc, in_maps, core_ids=list(range(N_CORES)))

    outp = np.zeros((N, O, S), np.float32)
    for c in range(N_CORES):
        outp[c // 2] += res.results[c]["out"]
    outp += bp[None, :, None]
    return outp.reshape(N, O, HH, WW)


# revision 17
# speedup vs baseline: 1.2642x; 1.2642x over previous
"""Causal attention (dense transformer block) on 8 Trainium2 NeuronCores.

Problem: x (4, 256, 64, 64) fp32; 1x1-conv q/kv projections; 8-head causal
attention over S = 64*64 = 4096 flattened pixels (head_dim 32); output
projection.  Full inputs in, full output out.

Sharding: 8 cores = 4 batches x 2 head-groups (4 heads each).  Each core
computes q/k/v projections for its head group, flash-style causal attention
(scores kept transposed: k-positions on partitions, q-positions on free dim,
so softmax denominators come out of the AV matmul via an appended ones
column), and a partial output projection.  Host sums the two head-group
partials per batch and adds the output bias.

Engine split: PE does all matmuls (f32r, full rate at moving>=256); ScalarE
does ONLY the softmax exp (it is the 2nd-busiest engine and exp alone is
~230us); DVE does projections' PSUM evacuation + bias, causal masks, and
softmax normalization; Pool broadcasts the reciprocal rows; denominator rows
move PSUM->SBUF by DMA.  Rows run j-descending so the per-row normalization
chain hides under long rows.
"""

import math
from contextlib import ExitStack

import numpy as np

import concourse.bass as bass
import concourse.tile as tile
from concourse import bacc, mybir

N_CORES = 8
N, C, HH, WW = 4, 256, 64, 64
S = HH * WW            # 4096
E = 256                # q/k width
O = 256                # v/out width
H = 8                  # heads
HD = E // H            # 32 head dim
HG = 4                 # heads per core
P = 128                # partitions
QC = 512               # q-chunk (matmul moving free dim)
KT = 128               # k-tile (contraction block for AV)
NQ = S // QC           # 8 q-chunks
G = 4                  # windows per emission group

F32 = mybir.dt.float32
F32R = mybir.dt.float32r
BF16 = mybir.dt.bfloat16

# bf16 for matmul STATIONARY operands only (k-tiles, v_st, wq/wk/wp): on
# HW, LDWEIGHTS streams 2 cols/cycle for 16-bit vs ~0.5 for f32r.  Moving
# operands stay f32r (full-rate at width>=256), so PE row cost is unchanged.
STAT_BF16 = False
SDT = BF16 if STAT_BF16 else F32R


def build_kernel(reps=1):
    nc = bacc.Bacc("TRN2", target_bir_lowering=False, debug=False,
                   num_devices=N_CORES)

    # Per-core inputs (same shapes on every core, different data).
    xf = nc.dram_tensor("xf", (C, S), F32, kind="ExternalInput").ap()
    wqT = nc.dram_tensor("wqT", (C, P), F32, kind="ExternalInput").ap()
    wkT = nc.dram_tensor("wkT", (C, P), F32, kind="ExternalInput").ap()
    wvT = nc.dram_tensor("wvT", (C, O), F32, kind="ExternalInput").ap()
    wpT = nc.dram_tensor("wpT", (2, P, P), F32, kind="ExternalInput").ap()
    bq = nc.dram_tensor("bq", (P, 1), F32, kind="ExternalInput").ap()
    bk = nc.dram_tensor("bk", (P, 1), F32, kind="ExternalInput").ap()
    bv = nc.dram_tensor("bv", (1, P), F32, kind="ExternalInput").ap()
    masks = nc.dram_tensor("masks", (4, P, QC), F32, kind="ExternalInput").ap()
    out = nc.dram_tensor("out", (O, S), F32, kind="ExternalOutput").ap()

    with tile.TileContext(nc) as tc:
        with ExitStack() as ctx:
            _emit(ctx, tc, nc, xf, wqT, wkT, wvT, wpT, bq, bk, bv, masks, out,
                  reps=reps)

    nc.compile()
    return nc


def _row_units(j):
    """Causal k-tile units for one (head, j) row: (kt, c0, width).

    Clean tiles (kt < 4j) are full 512.  Diagonal tiles shrink to their
    causally-valid columns where f32r still runs full rate (width >= 256);
    the 384-wide t1 unit goes last so its partially-used psum bank ends the
    row and the exp span stays dense.
    """
    units = [(kt, 0, QC) for kt in range(4 * j)]
    units.append((4 * j, 0, 512))
    units.append((4 * j + 2, 256, 256))
    units.append((4 * j + 3, 256, 256))
    units.append((4 * j + 1, 128, 384))
    return units


def _row_windows(j):
    """Pack a row's units into psum windows of <= 3 banks.

    Returns [(units_with_offsets, exp_width, spare_banks)] where units are
    (kt, c0, off, width), matmul psum writes never cross a 512-col bank,
    and spare_banks counts unused trailing banks in the (last) window —
    interleaved projection matmuls ride there.
    """
    units = _row_units(j)
    banks = []
    cur, used = [], 0
    for (kt, c0, w) in units:
        if used + w > QC:
            banks.append((cur, used))
            cur, used = [], 0
        cur.append((kt, c0, used, w))
        used += w
    banks.append((cur, used))

    windows = []
    for b0 in range(0, len(banks), 3):
        grp = banks[b0:b0 + 3]
        wunits = []
        exp_w = 0
        for bi, (bunits, used) in enumerate(grp):
            for (kt, c0, off, w) in bunits:
                wunits.append((kt, c0, bi * QC + off, w))
            exp_w = bi * QC + used
        windows.append((wunits, exp_w, 3 - len(grp)))
    return windows


def _emit(ctx, tc, nc, xf, wqT, wkT, wvT, wpT, bq, bk, bv, masks, out,
          reps=1):
    scale = 1.0 / math.sqrt(HD)
    Exp = mybir.ActivationFunctionType.Exp

    consts = ctx.enter_context(tc.tile_pool(name="consts", bufs=1))
    qk_ps = ctx.enter_context(tc.tile_pool(name="qk_ps", bufs=2, space="PSUM"))
    av_ps = ctx.enter_context(tc.tile_pool(name="av_ps", bufs=2, space="PSUM"))
    work = ctx.enter_context(tc.tile_pool(name="work", bufs=6))
    norm = ctx.enter_context(tc.tile_pool(name="norm", bufs=2))
    tmp = ctx.enter_context(tc.tile_pool(name="tmp", bufs=1))

    # ---- load constants / weights -------------------------------------
    # DMA-loaded data cannot feed FP32r matmuls directly; a compute-engine
    # copy with float32r output performs the required rounding.
    wq_ld = tmp.tile([P, 2, P], F32, tag="w")
    nc.sync.dma_start(out=wq_ld, in_=wqT.rearrange("(c p) m -> p c m", p=P))
    wq_sb = consts.tile([P, 2, P], SDT)
    nc.vector.tensor_copy(wq_sb, wq_ld)
    wk_ld = tmp.tile([P, 2, P], F32, tag="w")
    nc.sync.dma_start(out=wk_ld, in_=wkT.rearrange("(c p) m -> p c m", p=P))
    wk_sb = consts.tile([P, 2, P], SDT)
    nc.vector.tensor_copy(wk_sb, wk_ld)
    wv_ld = tmp.tile([P, 2, O], F32, tag="w")
    nc.sync.dma_start(out=wv_ld, in_=wvT.rearrange("(c p) m -> p c m", p=P))
    wv_sb = consts.tile([P, 2, O], F32R)
    nc.vector.tensor_copy(wv_sb, wv_ld)
    wp_ld = tmp.tile([P, 2, P], F32, tag="w")
    nc.sync.dma_start(out=wp_ld, in_=wpT.rearrange("m p n -> p m n"))
    wp_sb = consts.tile([P, 2, P], SDT)
    nc.vector.tensor_copy(wp_sb, wp_ld)
    # small tensors (biases, masks) go on the SP queue ahead of the big x
    # transfer so nothing downstream waits on queue order
    bq_sb = consts.tile([P, 1], F32)
    nc.sync.dma_start(out=bq_sb, in_=bq)
    bk_sb = consts.tile([P, 1], F32)
    nc.sync.dma_start(out=bk_sb, in_=bk)
    bv_row = consts.tile([1, P], F32)
    nc.sync.dma_start(out=bv_row, in_=bv)
    mask_sb = consts.tile([P, 4, QC], F32)
    nc.sync.dma_start(out=mask_sb, in_=masks.rearrange("m p q -> p m q"))
    # sliced load+round so the first projection matmuls start after one
    # 512-col slice instead of the whole 4 MB x transfer; Pool queue (idle,
    # 25ns dispatch) so the x slices don't serialize behind the SP queue
    x_ld = tmp.tile([P, 2, S], F32, tag="big")  # xf as two 128-row chunks
    x_sb = consts.tile([P, 2, S], F32R)
    xr = xf.rearrange("(c p) s -> p c s", p=P)
    for sl in range(NQ):
        nc.sync.dma_start(out=x_ld[:, :, bass.ts(sl, QC)],
                          in_=xr[:, :, bass.ts(sl, QC)])
        nc.vector.tensor_copy(x_sb[:, :, bass.ts(sl, QC)],
                              x_ld[:, :, bass.ts(sl, QC)])
    bv_bc = consts.tile([P, P], F32)            # bv broadcast down partitions
    nc.gpsimd.partition_broadcast(bv_bc, bv_row)

    # v_st[h]: (128 kpos, 34) per k-tile: cols 0:32 = v, col 32 = 1.0 (the
    # softmax-denominator ones column).  Ones written once, outside reps.
    v_st = [consts.tile([P, S // KT, 34], SDT, name=f"v_st{h}")
            for h in range(HG)]
    ones2 = consts.tile([P, 2], F32)
    nc.vector.memset(ones2, 1.0)
    ones_bc = bass.AP(tensor=ones2.tensor, offset=ones2.offset,
                      ap=[ones2.ap[0], [0, S // KT], ones2.ap[1]])
    for h in range(HG):
        nc.vector.tensor_copy(v_st[h][:, :, 32:34], ones_bc)

    # On-device repeat loop for timing runs (reps>1): the whole compute
    # phase re-executes; consts/DMA loads stay outside.
    if reps > 1:
        loop_cm = tc.For_i(0, reps, 1)
        loop_cm.__enter__()

    qT = consts.tile([P, S], F32R)               # 4 heads x 32 dims on parts
    kT = consts.tile([P, S], SDT)
    q3k3 = tmp.tile([HD, 2, S], F32R, tag="big")
    q3 = q3k3[:, 0, :]
    k3 = q3k3[:, 1, :]
    outn = consts.tile([P, S], F32R)             # normalized out^T, 4h x 32dv

    # ---- interleaved projection units -----------------------------------
    # Attention row (h, j) needs q/k projection columns jj <= j and v tiles
    # st <= 4j+3, so projections stream into the attention pipeline instead
    # of serializing up front.  They ride in the SPARE trailing psum banks
    # of each row's last window (no extra psum, no window-pool theft); each
    # unit needs exactly one 512-col bank.  j0/j1 needs run in a short
    # preamble (j0's window has no spares).

    bq_bc = bass.AP(tensor=bq_sb.tensor, offset=bq_sb.offset,
                    ap=[bq_sb.ap[0], [0, QC]])
    bk_bc = bass.AP(tensor=bk_sb.tensor, offset=bk_sb.offset,
                    ap=[bk_sb.ap[0], [0, QC]])

    def emit_qkproj(ps, off, arg):
        r, jj = arg
        dst, w_sb, b_bc = ((qT, wq_sb, bq_bc), (kT, wk_sb, bk_bc))[r]
        for cc in range(2):
            nc.tensor.matmul(ps[:, off:off + QC], w_sb[:, cc, :],
                             x_sb[:, cc, bass.ts(jj, QC)],
                             start=(cc == 0), stop=(cc == 1))
        nc.vector.tensor_add(dst[:, bass.ts(jj, QC)], ps[:, off:off + QC],
                             b_bc)
        # head 3 cannot feed PE weights from partition 96; partition-0 copy
        h3 = q3 if r == 0 else k3
        nc.vector.tensor_copy(h3[:, bass.ts(jj, QC)],
                              dst[3 * HD:4 * HD, bass.ts(jj, QC)])

    def emit_vproj(ps, off, st0):
        # position-major v, two k-tiles of 128 positions in one bank;
        # N=256 (all 8 heads) keeps f32r full-rate; wvT pre-rotated so our
        # 128 cols sit at 0:128 of each k-tile's 256-col output.
        for i, st in enumerate((st0, st0 + 1)):
            for cc in range(2):
                nc.tensor.matmul(ps[:, off + i * O:off + (i + 1) * O],
                                 x_sb[:, cc, bass.ts(st, KT)],
                                 wv_sb[:, cc, :],
                                 start=(cc == 0), stop=(cc == 1))
        for i in range(2):
            for h in range(HG):
                nc.vector.tensor_add(v_st[h][:, st0 + i, 0:32],
                                     ps[:, off + i * O + h * HD:
                                        off + i * O + (h + 1) * HD],
                                     bv_bc[:, h * HD:(h + 1) * HD])

    def emit_outproj(ps, off, arg):
        m, jj = arg
        nc.tensor.matmul(ps[:, off:off + QC], wp_sb[:, m, :],
                         outn[:, bass.ts(jj, QC)], start=True, stop=True)
        ob = work.tile([P, QC], F32, tag="ob", bufs=2)
        nc.vector.tensor_copy(ob, ps[:, off:off + QC])
        nc.sync.dma_start(
            out=out.rearrange("(m p) s -> p m s", p=P)[:, m,
                                                       bass.ts(jj, QC)],
            in_=ob)

    # ballast queue: (fn, arg), one psum bank each, in need order;
    # j0+j1 needs go to the preamble list instead
    preamble, ballast = [], []
    for jj in range(NQ):
        tgt = preamble if jj < 2 else ballast
        tgt.append((emit_qkproj, (0, jj)))
        tgt.append((emit_qkproj, (1, jj)))
        for st in range(4 * jj, 4 * jj + 4, 2):
            tgt.append((emit_vproj, st))

    for u0 in range(0, len(preamble), 3):
        ps = qk_ps.tile([P, 3 * QC], F32, tag="qk", name="pre")
        for bi, (fn, arg) in enumerate(preamble[u0:u0 + 3]):
            fn(ps, bi * QC, arg)

    # ---- attention ------------------------------------------------------
    # Rows (h, j) run j-ascending.  Windows emit in groups of G: G windows
    # of QK+exp+mask, then those windows' AV matmuls — long PE runs of one
    # stationary shape, with exp overlapping inside the group.  Projection
    # ballast rides spare banks; the per-row normalization chain (den copy
    # -> recip -> broadcast -> multiply) hides under following rows.
    flat = []                            # (h, j, wunits, exp_w, spare, u0, nu)
    for j in range(NQ):
        for h in range(HG):
            wins = _row_windows(j)
            nu = sum(len(wu) for (wu, _, _) in wins)
            u0 = 0
            for (wu, exp_w, spare) in wins:
                flat.append((h, j, wu, exp_w, spare, u0, nu))
                u0 += len(wu)
    groups = [flat[g0:g0 + G] for g0 in range(0, len(flat), G)]

    av_tiles = {}
    out_queue = []
    norm_done = set()

    def emit_qk_grp(grp):
        ess = []
        for (h, j, wunits, exp_w, spare, u0, nu) in grp:
            qh = qT[h * HD:(h + 1) * HD, :] if h < 3 else q3
            kh = kT[h * HD:(h + 1) * HD, :] if h < 3 else k3
            ps = qk_ps.tile([P, 3 * QC], F32, tag="qk", name="ps")
            for (kt, c0, off, w) in wunits:
                nc.tensor.matmul(ps[:, off:off + w],
                                 kh[:, bass.ts(kt, KT)],
                                 qh[:, j * QC + c0:(j + 1) * QC],
                                 start=True, stop=True)
            # projection ballast into this window's spare trailing banks
            for sp in range(spare):
                if ballast:
                    fn, arg = ballast.pop(0)
                elif out_queue:
                    fn, arg = out_queue.pop(0)
                else:
                    break
                fn(ps, (3 - spare + sp) * QC, arg)
            es = work.tile([P, 3 * QC], F32R, tag="es", name="es", bufs=5)
            nc.scalar.activation(es[:, 0:exp_w], ps[:, 0:exp_w],
                                 Exp, scale=scale)
            # causal mask post-exp (multiply by 0/1): off the QK->exp path
            for (kt, c0, off, w) in wunits:
                t = kt - 4 * j
                if t >= 0:
                    nc.vector.tensor_mul(es[:, off:off + w],
                                         es[:, off:off + w],
                                         mask_sb[:, t, c0:QC])
            ess.append(es)
        return ess

    def emit_av_grp(grp, ess):
        for es, (h, j, wunits, exp_w, spare, u0, nu) in zip(ess, grp):
            if (h, j) not in av_tiles:
                av_tiles[(h, j)] = av_ps.tile([P, QC], F32, tag="av",
                                              name="av")
            av = av_tiles[(h, j)]
            for ui, (kt, c0, off, w) in enumerate(wunits):
                nc.tensor.matmul(av[0:33, c0:QC],
                                 v_st[h][:, kt, 0:33],
                                 es[:, off:off + w],
                                 start=(u0 + ui == 0),
                                 stop=(u0 + ui == nu - 1))
            if u0 + len(wunits) == nu:
                # row complete: denominator -> reciprocal -> broadcast ->
                # normalize straight out of PSUM.
                den = norm.tile([1, QC], F32, tag="den", name="den")
                nc.vector.tensor_copy(den, av[32:33, :])
                recip = norm.tile([1, QC], F32, tag="recip", name="recip")
                rscr = norm.tile([1, QC], F32, tag="rscr", name="rscr")
                nc.vector.reciprocal_approx_accurate(recip, den, rscr)
                rbc = norm.tile([HD, QC], F32, tag="rbc", name="rbc")
                nc.gpsimd.partition_broadcast(rbc, recip)
                nc.vector.tensor_mul(
                    outn[h * HD:(h + 1) * HD, bass.ts(j, QC)],
                    av[0:32, :], rbc)
                del av_tiles[(h, j)]
                norm_done.add((h, j))
                if all((hh, j) in norm_done for hh in range(HG)):
                    for m in range(2):
                        out_queue.append((emit_outproj, (m, j)))

    for grp in groups:
        ess = emit_qk_grp(grp)
        emit_av_grp(grp, ess)

    # tail: whatever ballast/out-projection units found no spare bank
    rest = ballast + out_queue
    for u0 in range(0, len(rest), 3):
        ps = qk_ps.tile([P, 3 * QC], F32, tag="qk", name="tail")
        for bi, (fn, arg) in enumerate(rest[u0:u0 + 3]):
            fn(ps, bi * QC, arg)

    if reps > 1:
        loop_cm.__exit__(None, None, None)


_BUILT = {}


def _get_built(reps=1):
    if reps not in _BUILT:
        _BUILT[reps] = build_kernel(reps)
    return _BUILT[reps]


def make_in_maps(x, Wq, bq, Wkv, bkv, Wp, bp):
    x = np.asarray(x, dtype=np.float32)
    Wq = np.asarray(Wq, dtype=np.float32)
    bq = np.asarray(bq, dtype=np.float32)
    Wkv = np.asarray(Wkv, dtype=np.float32)
    bkv = np.asarray(bkv, dtype=np.float32)
    Wp = np.asarray(Wp, dtype=np.float32)

    Wk, Wv = Wkv[:E], Wkv[E:]
    bk_, bv_ = bkv[:E], bkv[E:]

    # causal masks in transposed-score orientation (kpos partition, qpos free)
    kk = np.arange(P)[:, None]
    qq = np.arange(QC)[None, :]
    mask_np = np.stack([
        (qq >= d0 + kk).astype(np.float32)
        for d0 in (0, 128, 256, 384)])

    in_maps = []
    for c in range(N_CORES):
        n, hg = c // 2, c % 2
        rows = slice(hg * P, (hg + 1) * P)
        # rotate wvT columns so this core's 128 head columns sit at 0:128
        wvT_c = np.ascontiguousarray(np.roll(Wv.T, -hg * P, axis=1))
        in_maps.append({
            "xf": np.ascontiguousarray(x[n].reshape(C, S)),
            "wqT": np.ascontiguousarray(Wq[rows].T),
            "wkT": np.ascontiguousarray(Wk[rows].T),
            "wvT": wvT_c,
            "wpT": np.ascontiguousarray(
                Wp[:, rows].reshape(2, P, P).transpose(0, 2, 1)),
            "bq": np.ascontiguousarray(bq[rows, None]),
            "bk": np.ascontiguousarray(bk_[rows, None]),
            "bv": np.ascontiguousarray(bv_[None, rows]),
            "masks": mask_np,
        })
    return in_maps


def kernel(x, Wq, bq, Wkv, bkv, Wp, bp, n_heads):
    assert int(n_heads) == H
    bp = np.asarray(bp, dtype=np.float32)

    from concourse.bass_utils import run_bass_kernel_spmd

    nc = _get_built()
    in_maps = make_in_maps(x, Wq, bq, Wkv, bkv, Wp, bp)

    res = run_bass_kernel_spmd(nc, in_maps, core_ids=list(range(N_CORES)))

    outp = np.zeros((N, O, S), np.float32)
    for c in range(N_CORES):
        outp[c // 2] += res.results[c]["out"]
    outp += bp[None, :, None]
    return outp.reshape(N, O, HH, WW)


# revision 21
# speedup vs baseline: 1.5199x; 1.2023x over previous
"""Causal attention (dense transformer block) on 8 Trainium2 NeuronCores.

Problem: x (4, 256, 64, 64) fp32; 1x1-conv q/kv projections; 8-head causal
attention over S = 64*64 = 4096 flattened pixels (head_dim 32); output
projection.  Full inputs in, full output out.

Sharding: 8 cores = 4 batches x 2 head-groups (4 heads each).  Each core
computes q/k/v projections for its head group, flash-style causal attention
(scores kept transposed: k-positions on partitions, q-positions on free dim,
so softmax denominators come out of the AV matmul via an appended ones
column), and a partial output projection.  Host sums the two head-group
partials per batch and adds the output bias.

All matmuls run as float32r (full PE rate at N>=512, near-fp32 precision).
Softmax skips the max-subtraction pass (scores are O(1) here, exp cannot
overflow) and normalizes after the AV matmul.
"""

import math
from contextlib import ExitStack

import numpy as np

import concourse.bass as bass
import concourse.tile as tile
from concourse import bacc, mybir

N_CORES = 8
N, C, HH, WW = 4, 256, 64, 64
S = HH * WW            # 4096
E = 256                # q/k width
O = 256                # v/out width
H = 8                  # heads
HD = E // H            # 32 head dim
HG = 4                 # heads per core
P = 128                # partitions
QC = 512               # q-chunk (matmul moving free dim)
KT = 128               # k-tile (contraction block for AV)
NQ = S // QC           # 8 q-chunks
NEGM = -1.0e5          # additive mask value (exp(-big) == 0)
ACT_W = 3              # k-tiles exp'd per ScalarE call (3 psum banks)

F32 = mybir.dt.float32
F32R = mybir.dt.float32r
BF16 = mybir.dt.bfloat16

# QK scores in bf16: stationary loads 2 cols/cycle (vs ~0.5 for fp32r), the
# single biggest per-matmul cost in the K=32 QK shape.  Rel-err impact is
# ~2e-3 (scores are O(1) logits; softmax renormalizes).
QK_BF16 = False
QK_DT = BF16 if QK_BF16 else F32R


def build_kernel(reps=1):
    nc = bacc.Bacc("TRN2", target_bir_lowering=False, debug=False,
                   num_devices=N_CORES)

    # Per-core inputs (same shapes on every core, different data).
    xf = nc.dram_tensor("xf", (C, S), F32, kind="ExternalInput").ap()
    wqT = nc.dram_tensor("wqT", (C, P), F32, kind="ExternalInput").ap()
    wkT = nc.dram_tensor("wkT", (C, P), F32, kind="ExternalInput").ap()
    wvT = nc.dram_tensor("wvT", (C, O), F32, kind="ExternalInput").ap()
    wpT = nc.dram_tensor("wpT", (2, P, P), F32, kind="ExternalInput").ap()
    bq = nc.dram_tensor("bq", (P, 1), F32, kind="ExternalInput").ap()
    bk = nc.dram_tensor("bk", (P, 1), F32, kind="ExternalInput").ap()
    bv = nc.dram_tensor("bv", (1, P), F32, kind="ExternalInput").ap()
    masks = nc.dram_tensor("masks", (4, P, QC), F32, kind="ExternalInput").ap()
    out = nc.dram_tensor("out", (O, S), F32, kind="ExternalOutput").ap()

    with tile.TileContext(nc) as tc:
        with ExitStack() as ctx:
            _emit(ctx, tc, nc, xf, wqT, wkT, wvT, wpT, bq, bk, bv, masks, out,
                  reps=reps)

    nc.compile()
    return nc


def _emit(ctx, tc, nc, xf, wqT, wkT, wvT, wpT, bq, bk, bv, masks, out,
          reps=1):
    scale = 1.0 / math.sqrt(HD)
    Exp = mybir.ActivationFunctionType.Exp
    Ident = mybir.ActivationFunctionType.Identity

    consts = ctx.enter_context(tc.tile_pool(name="consts", bufs=1))
    qk_ps = ctx.enter_context(tc.tile_pool(name="qk_ps", bufs=2, space="PSUM"))
    av_ps = ctx.enter_context(tc.tile_pool(name="av_ps", bufs=2, space="PSUM"))
    work = ctx.enter_context(tc.tile_pool(name="work", bufs=6))
    norm = ctx.enter_context(tc.tile_pool(name="norm", bufs=2))
    tmp = ctx.enter_context(tc.tile_pool(name="tmp", bufs=1))

    # ---- load constants / weights -------------------------------------
    # DMA-loaded data cannot feed FP32r matmuls directly; a compute-engine
    # copy with float32r output performs the required rounding.
    wq_ld = tmp.tile([P, 2, P], F32, tag="w")
    nc.sync.dma_start(out=wq_ld, in_=wqT.rearrange("(c p) m -> p c m", p=P))
    wq_sb = consts.tile([P, 2, P], F32R)
    nc.vector.tensor_copy(wq_sb, wq_ld)
    wk_ld = tmp.tile([P, 2, P], F32, tag="w")
    nc.sync.dma_start(out=wk_ld, in_=wkT.rearrange("(c p) m -> p c m", p=P))
    wk_sb = consts.tile([P, 2, P], F32R)
    nc.vector.tensor_copy(wk_sb, wk_ld)
    wv_ld = tmp.tile([P, 2, O], F32, tag="w")
    nc.sync.dma_start(out=wv_ld, in_=wvT.rearrange("(c p) m -> p c m", p=P))
    wv_sb = consts.tile([P, 2, O], F32R)
    nc.vector.tensor_copy(wv_sb, wv_ld)
    wp_ld = tmp.tile([P, 2, P], F32, tag="w")
    nc.sync.dma_start(out=wp_ld, in_=wpT.rearrange("m p n -> p m n"))
    wp_sb = consts.tile([P, 2, P], F32R)
    nc.vector.tensor_copy(wp_sb, wp_ld)
    # small tensors (biases, masks) ahead of the big x transfer so nothing
    # downstream waits on SP queue order
    bq_sb = consts.tile([P, 1], F32)
    nc.sync.dma_start(out=bq_sb, in_=bq)
    bk_sb = consts.tile([P, 1], F32)
    nc.sync.dma_start(out=bk_sb, in_=bk)
    bv_row = consts.tile([1, P], F32)
    nc.sync.dma_start(out=bv_row, in_=bv)
    mask_sb = consts.tile([P, 4, QC], F32)
    nc.sync.dma_start(out=mask_sb, in_=masks.rearrange("m p q -> p m q"))
    # sliced load+round so the first projection matmuls start after one
    # 512-col slice instead of the whole 4 MB x transfer (~19us startup)
    x_ld = tmp.tile([P, 2, S], F32, tag="big")  # xf as two 128-row chunks
    x_sb = consts.tile([P, 2, S], F32R)
    xr = xf.rearrange("(c p) s -> p c s", p=P)
    for sl in range(NQ):
        nc.sync.dma_start(out=x_ld[:, :, bass.ts(sl, QC)],
                          in_=xr[:, :, bass.ts(sl, QC)])
        nc.vector.tensor_copy(x_sb[:, :, bass.ts(sl, QC)],
                              x_ld[:, :, bass.ts(sl, QC)])

    bv_bc = consts.tile([P, P], F32)            # bv broadcast down partitions
    nc.gpsimd.partition_broadcast(bv_bc, bv_row)

    # On-device repeat loop for timing runs (reps>1): the whole compute
    # phase re-executes; consts/DMA loads stay outside.
    if reps > 1:
        loop_cm = tc.For_i(0, reps, 1)
        loop_cm.__enter__()

    # ---- q/k projections: qT/kT = W.T-slice @ xf + bias ----------------
    qT = consts.tile([P, S], QK_DT)              # 4 heads x 32 dims on partitions
    kT = consts.tile([P, S], QK_DT)
    for dst, w_sb, b_sb in ((qT, wq_sb, bq_sb), (kT, wk_sb, bk_sb)):
        for j in range(NQ):
            ps = qk_ps.tile([P, 3 * QC], F32, tag="qk")
            for cc in range(2):
                nc.tensor.matmul(ps[:, 0:QC], w_sb[:, cc, :],
                                 x_sb[:, cc, bass.ts(j, QC)],
                                 start=(cc == 0), stop=(cc == 1))
            nc.scalar.activation(dst[:, bass.ts(j, QC)], ps[:, 0:QC],
                                 Ident, bias=b_sb, scale=1.0)

    # ---- v projection, position-major: v[s, o] for our 4 heads ---------
    # One k-tile of 128 positions per matmul; N=256 (all 8 heads) keeps
    # float32r at full rate; we keep only our head-group's 128 columns.
    # v_st[h]: (128 kpos, 34) per k-tile: cols 0:32 = v, col 32 = 1.0.
    v_st = [consts.tile([P, S // KT, 34], F32R, name=f"v_st{h}")
            for h in range(HG)]
    ones2 = consts.tile([P, 2], F32)
    nc.vector.memset(ones2, 1.0)
    ones_bc = bass.AP(tensor=ones2.tensor, offset=ones2.offset,
                      ap=[ones2.ap[0], [0, S // KT], ones2.ap[1]])
    for h in range(HG):
        nc.vector.tensor_copy(v_st[h][:, :, 32:34], ones_bc)
    # wvT columns are pre-rotated on the host so this core's head-group
    # occupies columns 0:128 of the v projection output.
    for st0 in range(0, S // KT, 6):
        cnt = min(6, S // KT - st0)
        ps = qk_ps.tile([P, 6, O], F32, tag="qk")
        for i in range(cnt):
            for cc in range(2):
                nc.tensor.matmul(ps[:, i, :],
                                 x_sb[:, cc, bass.ts(st0 + i, KT)],
                                 wv_sb[:, cc, :], start=(cc == 0),
                                 stop=(cc == 1))
        for h in range(HG):
            bv3 = bass.AP(tensor=bv_bc.tensor,
                          offset=bv_bc.offset + h * HD,
                          ap=[bv_bc.ap[0], [0, cnt], [1, HD]])
            nc.vector.tensor_add(v_st[h][:, st0:st0 + cnt, 0:32],
                                 ps[:, 0:cnt, h * HD:(h + 1) * HD],
                                 bv3)

    # Matmul operands cannot start at partition 96 (PE quadrant-3 weight
    # feed is unsupported), so head 3's q/k rows get their own partition-0
    # tiles.
    q3k3 = tmp.tile([HD, 2, S], QK_DT, tag="big")
    nc.vector.tensor_copy(q3k3[:, 0, :], qT[3 * HD:4 * HD, :])
    nc.vector.tensor_copy(q3k3[:, 1, :], kT[3 * HD:4 * HD, :])

    # ---- attention ------------------------------------------------------
    # Emission in window-groups of G: G windows of QK+exp(+mask), then those
    # windows' AV matmuls.  Grouping keeps the PE on long runs of one
    # stationary shape (QK vs AV weight loads serialize when alternating),
    # and the one-group skew lets ScalarE exp run concurrently with both.
    # The softmax denominator row is copied out of PSUM immediately so the
    # accumulator bank frees before the (serial, DVE) normalization chain.
    G = 4
    outn = consts.tile([P, S], F32R)             # normalized out^T, 4h x 32dv
    # Pack k-tiles into psum windows by column width (<= 1536).  Diagonal
    # tiles of chunks j>=1 shrink to their causally-valid columns [c0:512]
    # (c0 capped at 256 for full-rate fp32r), packed densely so the exp
    # window is one fully-written contiguous span.
    # Matmul psum writes cannot cross a 512-col bank: full tiles take one
    # bank; the two shrunk (256-wide) diagonal tiles pair into one bank.
    # Per row: clean k-tiles full 512; diagonal tiles shrink to their
    # causally-valid columns (t0: 512, t1: 384 at c0=128, t2/t3: 256 at
    # c0=256).  Banks: clean singles, then [t0][t2+t3][t1-last] so the
    # partially-used 384 bank ends the row and every exp span is dense.
    windows = []                                 # (h, j, nkt, [(kt, c0, off)])
    for h in range(HG):
        for j in range(NQ):
            nkt = 4 * j + 4
            units = [(kt, 0) for kt in range(4 * j)]
            units += [(4 * j, 0), (4 * j + 2, 256), (4 * j + 3, 256),
                      (4 * j + 1, 128)]
            banks = []
            cur, used = [], 0
            ui = 0
            for (kt, c0) in units:
                w = QC - c0
                if used + w > QC:
                    banks.append(cur)
                    cur, used = [], 0
                cur.append((kt, c0, used, ui))
                used += w
                ui += 1
            banks.append(cur)
            for b0 in range(0, len(banks), 3):
                wunits = []
                for bi, bunits in enumerate(banks[b0:b0 + 3]):
                    for (kt, c0, off, ui) in bunits:
                        wunits.append((kt, c0, bi * QC + off, ui))
                windows.append((h, j, nkt, wunits))
    groups = [windows[g0:g0 + G] for g0 in range(0, len(windows), G)]

    av_tiles = {}

    def emit_qk_grp(grp):
        ess = []
        for (h, j, nkt, tiles) in grp:
            qh = qT[h * HD:(h + 1) * HD, :] if h < 3 else q3k3[:, 0, :]
            kh = kT[h * HD:(h + 1) * HD, :] if h < 3 else q3k3[:, 1, :]
            ps = qk_ps.tile([P, 3 * QC], F32, tag="qk", name="ps")
            width = 0
            for (kt, c0, off, ui) in tiles:
                width = max(width, off + QC - c0)
                nc.tensor.matmul(ps[:, off:off + QC - c0],
                                 kh[:, bass.ts(kt, KT)],
                                 qh[:, j * QC + c0:(j + 1) * QC],
                                 start=True, stop=True)
            es = work.tile([P, ACT_W * QC], F32R, tag="es", name="es", bufs=5)
            nc.scalar.activation(es[:, 0:width], ps[:, 0:width],
                                 Exp, scale=scale)
            # causal mask post-exp (multiply by 0/1): off the QK->exp path
            for (kt, c0, off, ui) in tiles:
                if kt >= 4 * j:
                    sl = es[:, off:off + QC - c0]
                    nc.vector.tensor_mul(sl, sl,
                                         mask_sb[:, kt - 4 * j, c0:QC])
            ess.append(es)
        return ess

    def emit_av_grp(grp, ess):
        for es, (h, j, nkt, tiles) in zip(ess, grp):
            if (h, j) not in av_tiles:
                av_tiles[(h, j)] = av_ps.tile([33, QC], F32, tag="av",
                                              name="av")
            av = av_tiles[(h, j)]
            for (kt, c0, off, ui) in tiles:
                nc.tensor.matmul(av[:, c0:QC], v_st[h][:, kt, 0:33],
                                 es[:, off:off + QC - c0],
                                 start=(ui == 0), stop=(ui == nkt - 1))
            if tiles[-1][3] == nkt - 1:
                # quick PSUM evacuation, then normalize rows 0:32 by row 32.
                avs = norm.tile([32, QC], F32, tag="avs", name="avs")
                nc.vector.tensor_copy(avs, av[0:32, :])
                l0 = norm.tile([1, QC], F32, tag="l0", name="l0")
                nc.vector.tensor_copy(l0, av[32:33, :])
                recip = norm.tile([1, QC], F32, tag="recip", name="recip")
                rscr = norm.tile([1, QC], F32, tag="rscr", name="rscr", bufs=1)
                nc.vector.reciprocal_approx_accurate(recip, l0, rscr)
                rbc = norm.tile([32, QC], F32, tag="rbc", name="rbc")
                nc.gpsimd.partition_broadcast(rbc, recip)
                nc.vector.tensor_mul(outn[h * HD:(h + 1) * HD, bass.ts(j, QC)],
                                     avs, rbc)
                del av_tiles[(h, j)]

    for grp in groups:
        ess = emit_qk_grp(grp)
        emit_av_grp(grp, ess)

    # ---- output projection: out = Wp[:, our 128 cols] @ outn ----------
    for j in range(NQ):
        for m in range(2):
            ps = qk_ps.tile([P, 3 * QC], F32, tag="qk")
            nc.tensor.matmul(ps[:, 0:QC], wp_sb[:, m, :],
                             outn[:, bass.ts(j, QC)],
                             start=True, stop=True)
            ob = work.tile([P, QC], F32, tag="ob", bufs=4)
            nc.scalar.activation(ob, ps[:, 0:QC], Ident, bias=0.0, scale=1.0)
            nc.sync.dma_start(
                out=out.rearrange("(m p) s -> p m s", p=P)[:, m,
                                                           bass.ts(j, QC)],
                in_=ob)

    if reps > 1:
        loop_cm.__exit__(None, None, None)


_BUILT = {}


def _get_built(reps=1):
    if reps not in _BUILT:
        _BUILT[reps] = build_kernel(reps)
    return _BUILT[reps]


def make_in_maps(x, Wq, bq, Wkv, bkv, Wp, bp):
    x = np.asarray(x, dtype=np.float32)
    Wq = np.asarray(Wq, dtype=np.float32)
    bq = np.asarray(bq, dtype=np.float32)
    Wkv = np.asarray(Wkv, dtype=np.float32)
    bkv = np.asarray(bkv, dtype=np.float32)
    Wp = np.asarray(Wp, dtype=np.float32)

    Wk, Wv = Wkv[:E], Wkv[E:]
    bk_, bv_ = bkv[:E], bkv[E:]

    # causal masks in transposed-score orientation (kpos partition, qpos free)
    kk = np.arange(P)[:, None]
    qq = np.arange(QC)[None, :]
    mask_np = np.stack([
        (qq >= d0 + kk).astype(np.float32)
        for d0 in (0, 128, 256, 384)])

    in_maps = []
    for c in range(N_CORES):
        n, hg = c // 2, c % 2
        rows = slice(hg * P, (hg + 1) * P)
        # rotate wvT columns so this core's 128 head columns sit at 0:128
        wvT_c = np.ascontiguousarray(np.roll(Wv.T, -hg * P, axis=1))
        in_maps.append({
            "xf": np.ascontiguousarray(x[n].reshape(C, S)),
            "wqT": np.ascontiguousarray(Wq[rows].T),
            "wkT": np.ascontiguousarray(Wk[rows].T),
            "wvT": wvT_c,
            "wpT": np.ascontiguousarray(
                Wp[:, rows].reshape(2, P, P).transpose(0, 2, 1)),
            "bq": np.ascontiguousarray(bq[rows, None]),
            "bk": np.ascontiguousarray(bk_[rows, None]),
            "bv": np.ascontiguousarray(bv_[None, rows]),
            "masks": mask_np,
        })
    return in_maps


def kernel(x, Wq, bq, Wkv, bkv, Wp, bp, n_heads):
    assert int(n_heads) == H
    bp = np.asarray(bp, dtype=np.float32)

    from concourse.bass_utils import run_bass_kernel_spmd

    nc = _get_built()
    in_maps = make_in_maps(x, Wq, bq, Wkv, bkv, Wp, bp)

    res = run_bass_kernel_spmd(nc, in_maps, core_ids=list(range(N_CORES)))

    outp = np.zeros((N, O, S), np.float32)
    for c in range(N_CORES):
        outp[c // 2] += res.results[c]["out"]
    outp += bp[None, :, None]
    return outp.reshape(N, O, HH, WW)

